# revision 1
# baseline (speedup 1.0000x reference)
"""GCNConv (batched dense-adjacency GraphConv) Trainium2 kernel.

Math: out[b] = sum_n relu((A[b] @ X[b]) @ W1 + b1) @ W2 + N * b2
Using (A X) W1 == A (X W1), precompute Y = X @ W1 on host (tiny), so the
device only does the memory-bound part: Z^T = Y^T A^T per batch, then
relu+bias and a node-sum. Host finishes with the [B,4] @ [4,1] readout.

Default strategy "f8ms" (fp8 mean-shift): since A ~ U[0,1), the device
streams D = A - 0.5 in float8e4 (e4m3, 1 byte/elem — the DMA of A is
the roofline), and the exact mean term 0.5*colsum(Y) is folded into a
per-batch bias computed on the host in fp64. That cancellation is what
makes a single e4m3 copy of Y accurate enough (rel err ~7e-4 vs the
2e-2 gate; without the shift it is ~2e-2). Per batch the PE runs two
DoubleRow fp8 matmuls (contraction chunk pairs, 0.5 cycles/moving row),
the Act engine applies relu+bias into a small bf16 scratch, and the DVE
computes the node-sum via bn_stats (sum = 256*(mean_even + mean_odd) on
the host side; ACT's accum_out reader costs ~280ns/op on TRN2, bn_stats
moves that work to the otherwise idle DVE).

Sharding: pure data parallel over the batch dim, 32 batches per core x 8.
The older multi-precision stream strategies (fp16s etc.) are kept below
for reference and can be selected via STRATEGY.
"""

import sys

if "/opt/trn_rl_repo" not in sys.path:
    sys.path.insert(0, "/opt/trn_rl_repo")

import numpy as np

import concourse.bass as bass  # noqa: F401
import concourse.mybir as mybir
import concourse.tile as tile
from concourse import bacc
from concourse.bass_utils import run_bass_kernel_spmd

N_CORES = 8
B, N, F = 256, 512, 2
H = 4  # hidden dim after W1
BPC = B // N_CORES  # batches per core
NCH = N // 128  # contraction chunks per batch

# Each strategy is a list of streams; stream = (a_dtype, a_part, y_comps)
#   a_dtype: dtype of the A^T moving tensor for this stream
#   a_part:  "hi" -> round(A), "lo" -> A - round_as(A, a_dtype of stream 0)
#   y_comps: stationary components stacked in lhsT, each (y_dtype, y_part)
# Error ~ (A rounding of finest stream) x (Y rounding of finest comp).
_F32 = "float32"
_F32R = "float32r"
_BF16 = "bfloat16"
_FP16 = "float16"

STRATEGIES = {
    # exact fp32 (PE at 1/4 rate)
    "fp32": [(_F32, "hi", [(_F32, "hi")])],
    # relaxed-precision full-rate fp32 matmul
    "f32r": [(_F32R, "hi", [(_F32R, "hi")])],
    # plain bf16 / fp16 (half DMA bytes)
    "bf16": [(_BF16, "hi", [(_BF16, "hi")])],
    "fp16": [(_FP16, "hi", [(_FP16, "hi")])],
    # fp16 A, Y split hi+lo (kills the Y-rounding term; same bytes as fp16)
    "fp16s": [(_FP16, "hi", [(_FP16, "hi"), (_FP16, "lo")])],
    # bf16 hi/lo split of A and Y: fp32-class accuracy, fp32 DMA bytes,
    # but only 2 moving passes of the PE per batch
    "bf16x2s": [
        (_BF16, "hi", [(_BF16, "hi"), (_BF16, "lo")]),
        (_BF16, "lo", [(_BF16, "hi")]),
    ],
}

STRATEGY = "f8ms"

TUNE = {}

_BUILT = {}


def _dt(name):
    return getattr(mybir.dt, name)


def _np_dt(name):
    import ml_dtypes
    return {"float32": np.float32, "float32r": np.float32,
            "bfloat16": ml_dtypes.bfloat16, "float16": np.float16}[name]


def _build_f8ms(repeat=1):
    """fp8 mean-shift strategy: A is stored as D = A - 0.5 in float8e4
    (e4m3); the exact 0.5*colsum(Y) mean term is folded into a per-batch
    bias on the host, so a single e4m3 Y component suffices (no PSUM band
    combine). Matmuls run in DoubleRow perf mode (2 contraction chunks per
    pass, 0.5 PE cycles per moving row)."""
    f32 = mybir.dt.float32
    f8 = mybir.dt.float8e4
    DR = mybir.MatmulPerfMode.DoubleRow

    nc = bacc.Bacc("TRN2", target_bir_lowering=False, debug=False,
                   num_devices=N_CORES,
                   num_swdge_queues=TUNE.get("swdge_queues", 1))

    # NOTE: DoubleRow matmuls may only write PSUM quadrant 0 (the ISA pins
    # LDWEIGHTS col_grp=0xf, and s3d3_mm_valid_dst_partition then requires
    # dst partition base 0), so batches cannot stack across PSUM quadrants.

    # at0[b]: D[b]^T packed [128, NCH, N]: at0[b][p][c][n] = D[b][n][c*128+p]
    # apack: partition-major [128, BPC*NCH*N] so a multi-batch DMA sees
    # long contiguous per-partition runs in DRAM (bigger descriptors)
    apack = TUNE.get("apack", False)
    if apack:
        atd = nc.dram_tensor("at0", [128, BPC * NCH * N], f8,
                             kind="ExternalInput")
    else:
        atd = nc.dram_tensor("at0", [BPC, 128, NCH, N], f8,
                             kind="ExternalInput")
    # y0[p][b*NCH+c][j] = Y[b][c*128+p][j]
    yd = nc.dram_tensor("y0", [128, BPC * NCH, H], f8, kind="ExternalInput")
    # b1s[j, b] = b1[j] + 0.5*colsum(Y[b])[j] for j < H, zeros for j >= H
    b1pd = nc.dram_tensor("b1p", [32, BPC], f32, kind="ExternalInput")
    # out[:, 6b:6b+6]: DVE bn_stats of batch b's relu'd rows (sum = 256 *
    # (mean_even + mean_odd)); the act_accum path writes col 6b only
    outd = nc.dram_tensor("out", [32, 6 * BPC], f32, kind="ExternalOutput")

    a_bufs = TUNE.get("a_bufs", 24)
    a_dma_split = TUNE.get("a_dma_split", 1)
    psum_bufs = TUNE.get("psum_bufs", 4)
    scratch_bufs = TUNE.get("scratch_bufs", 8)
    G = TUNE.get("group", 2)
    # DoubleRow LDWEIGHTS ISA: pair-dim stride must be % 16 == 0, so the
    # per-(b,c) Y block is spread on-chip from tight H columns to ypad.
    ypad = TUNE.get("ypad", 16)
    # stationary columns per pair (= PSUM output partitions); 4 keeps the
    # Act/DVE/PSUM footprint minimal (only rows 0..H-1 are ever read)
    mcols = TUNE.get("mcols", 4)

    with tile.TileContext(nc) as tc:
        with tc.tile_pool(name="const", bufs=1) as constp, \
             tc.tile_pool(name="apool", bufs=a_bufs) as apool, \
             tc.tile_pool(name="scratch", bufs=scratch_bufs) as spool, \
             tc.tile_pool(name="psum", bufs=psum_bufs, space="PSUM") as ppool:
            b1p_t = constp.tile([32, BPC], f32)
            nc.sync.dma_start(out=b1p_t[:], in_=b1pd[:])
            yt_t = constp.tile([128, BPC * NCH, H], f8)
            nc.sync.dma_start(out=yt_t[:], in_=yd[:])
            y_t = constp.tile([128, BPC * NCH, ypad], f8)
            nc.vector.memset(y_t[:], 0.0)
            nc.vector.tensor_copy(out=y_t[:, :, 0:H], in_=yt_t[:])
            out_t = constp.tile([32, 6 * BPC], f32)
            nc.vector.memset(out_t[:], 0.0)
            a_const = None
            if TUNE.get("no_adma"):
                a_const = constp.tile([128, NCH, N], f8)
                nc.vector.memset(a_const[:], 0.0)

            dma_only = TUNE.get("dma_only", False)  # debug: timing floor
            no_adma = TUNE.get("no_adma", False)  # debug: compute ceiling
            dma_engs = [getattr(nc, e)
                        for e in TUNE.get("dma_engs", ("sync",))]

            # apair: one dma_start covers `apair` consecutive batches (fewer
            # DMA instructions; same descriptor sizes)
            apair = TUNE.get("apair", 2)

            def fetch_a(b0, nb):
                a_t = apool.tile([128, nb, NCH, N], f8, tag="a0")
                if apack:
                    blk = NCH * N
                    nc.sync.dma_start(
                        out=a_t[:],
                        in_=atd[:, b0 * blk:(b0 + nb) * blk]
                        .rearrange("p (b c n) -> p b c n", b=nb, n=N))
                    return a_t
                step = NCH // a_dma_split
                for d in range(a_dma_split):
                    eng = dma_engs[(b0 * a_dma_split + d) % len(dma_engs)]
                    eng.dma_start(
                        out=a_t[:, :, d * step:(d + 1) * step, :],
                        in_=atd[b0:b0 + nb][:, :, d * step:(d + 1) * step, :]
                        .rearrange("b p c n -> p b c n"))
                return a_t

            no_act = TUNE.get("no_act", False)  # debug: DMA+PE only
            act_accum = TUNE.get("act_accum", False)  # accum_out on ActE
            # (default: relu+bias on ActE, node-sum as pool_avg on DVE —
            # the TRN2 Act accumulator read costs ~280ns per op)

            def group_body(b, a_t):
                ps = ppool.tile([mcols, N], f32)
                for c in range(0, NCH, 2):
                    nc.tensor.matmul(
                        ps[0:mcols, :],
                        y_t[:, b * NCH + c:b * NCH + c + 2, 0:mcols],
                        a_t[:, c:c + 2, :],
                        start=(c == 0), stop=(c == NCH - 2),
                        perf_mode=DR,
                        skip_group_check=True)
                if no_act:
                    return
                sc_dt = (mybir.dt.bfloat16 if TUNE.get("sc_bf16", True)
                         else f32)
                sc = spool.tile([mcols, N], sc_dt, tag="sc")
                if act_accum:
                    nc.scalar.activation(
                        sc[:], ps[0:mcols, :],
                        mybir.ActivationFunctionType.Relu,
                        bias=b1p_t[0:mcols, b:b + 1], scale=1.0,
                        accum_out=out_t[0:mcols, 6 * b:6 * b + 1])
                else:
                    nc.scalar.activation(
                        sc[:], ps[0:mcols, :],
                        mybir.ActivationFunctionType.Relu,
                        bias=b1p_t[0:mcols, b:b + 1], scale=1.0)
                    nc.vector.bn_stats(
                        out_t[0:mcols, 6 * b:6 * b + 6], sc[:])

            def emit_all():
                cache = {}
                for b in range(BPC):
                    if TUNE.get("empty"):
                        continue
                    if no_adma:
                        a_t = a_const[:, :, :]
                    else:
                        b0 = (b // apair) * apair
                        if b0 not in cache:
                            cache[b0] = fetch_a(b0, apair)
                        a_t = cache[b0][:, b - b0, :, :]
                    if dma_only:
                        continue
                    group_body(b, a_t)

            if repeat == 1:
                emit_all()
            else:
                hints = tuple(getattr(mybir.EngineType, e)
                              for e in TUNE.get("hint", ()))
                with tc.For_i(0, repeat, 1, hint_engines=hints,
                              staggered_reset=TUNE.get("staggered", False)):
                    emit_all()
            nc.sync.dma_start(out=outd[:], in_=out_t[:])

    nc.compile()
    return nc


def _build(strategy, repeat=1):
    """Build + compile the Bass module (once per process per strategy).

    repeat > 1 wraps the per-batch loop in a device-side For loop that
    re-runs the whole workload `repeat` times — used only for timing
    (amortizes host dispatch overhead over many on-device iterations).
    """
    if strategy == "f8ms":
        return _build_f8ms(repeat)
    streams = STRATEGIES[strategy]
    f32 = mybir.dt.float32

    nc = bacc.Bacc("TRN2", target_bir_lowering=False, debug=False,
                   num_devices=N_CORES,
                   num_swdge_queues=TUNE.get("swdge_queues", 1))

    # at_s[b]: A[b]^T packed [128, NCH*N]: at[b][p][c*N+n] = A_part[b][n][c*128+p]
    ats = [nc.dram_tensor(f"at{s}", [BPC, 128, NCH * N], _dt(a_dt),
                          kind="ExternalInput")
           for s, (a_dt, _, _) in enumerate(streams)]
    # y_s packed [128, BPC*NCH*W_s]; per (b,c) block comp k sits at
    # columns [32*k, 32*k+H) (zeros elsewhere)
    def _w(y_comps):
        return 32 * (len(y_comps) - 1) + H

    # y travels tight (ncomp*H per block); it is spread to the 32-col-spaced
    # lhsT layout on-chip (pad columns stay garbage — their PSUM rows are
    # never read)
    tight_y = TUNE.get("tight_y", True)

    def _wt(y_comps):
        return len(y_comps) * H if tight_y else _w(y_comps)

    ys = [nc.dram_tensor(f"y{s}", [128, BPC * NCH * _wt(y_comps)],
                         _dt(y_comps[0][0]), kind="ExternalInput")
          for s, (_, _, y_comps) in enumerate(streams)]
    b1d = nc.dram_tensor("b1", [H, 1], f32, kind="ExternalInput")
    outd = nc.dram_tensor("out", [H, BPC], f32, kind="ExternalOutput")

    # Engine APs must start at a partition base that's a multiple of 32, so
    # stream 0's stacked components live in 32-row PSUM bands (rows 32k..+H,
    # lhsT stacks them 32 columns apart with zero padding). Streams >= 1 must
    # be single-component; they accumulate straight into rows 0..H via the
    # PE's PSUM accumulation (start=False), costing no combine work.
    n_groups = len(streams[0][2])
    for (_, _, y_comps) in streams[1:]:
        assert len(y_comps) == 1, "secondary streams must be single-component"
    n_rows = 32 * (n_groups - 1) + H  # psum tile partition count

    # SBUF budget is ample; buffer enough A tiles to keep DMA queues busy.
    a_bufs = TUNE.get("a_bufs", 24)
    # number of dma_start's per A tile (more transfers -> more DMA queues)
    a_dma_split = TUNE.get("a_dma_split", 1)
    psum_bufs = TUNE.get("psum_bufs", 4)
    scratch_bufs = TUNE.get("scratch_bufs", 8)

    with tile.TileContext(nc) as tc:
        with tc.tile_pool(name="const", bufs=1) as constp, \
             tc.tile_pool(name="apool", bufs=a_bufs) as apool, \
             tc.tile_pool(name="scratch", bufs=scratch_bufs) as spool, \
             tc.tile_pool(name="psum", bufs=psum_bufs, space="PSUM") as ppool:
            b1_t = constp.tile([H, 1], f32)
            nc.sync.dma_start(out=b1_t[:], in_=b1d[:])
            y_ts = []
            for s, (_, _, y_comps) in enumerate(streams):
                w, wt = _w(y_comps), _wt(y_comps)
                dt_s = _dt(y_comps[0][0])
                y_t = constp.tile([128, BPC * NCH * w], dt_s, tag=f"y{s}")
                if w == wt:
                    nc.sync.dma_start(out=y_t[:], in_=ys[s][:])
                else:
                    yt_t = constp.tile([128, BPC * NCH * wt], dt_s,
                                       tag=f"yt{s}")
                    nc.sync.dma_start(out=yt_t[:], in_=ys[s][:])
                    nblk = BPC * NCH
                    dst3 = y_t[:].rearrange("p (blk w) -> p blk w", w=w)
                    src3 = yt_t[:].rearrange("p (blk w) -> p blk w", w=wt)
                    for k in range(len(y_comps)):
                        nc.vector.tensor_copy(
                            out=dst3[:, :, 32 * k:32 * k + H],
                            in_=src3[:, :, H * k:H * k + H])
                y_ts.append(y_t)
            out_t = constp.tile([H, BPC], f32)

            # G batches share one PSUM tile (side by side along the free dim,
            # one 2KB bank each) so the tiny combine/activation ops amortize
            # their per-op overhead over G*N columns.
            G = TUNE.get("group", 2)

            def group_body(b0, Gs=None):
                Gs = Gs or G
                ps = ppool.tile([n_rows, G * N], f32)
                last_s = len(streams) - 1
                for gi in range(Gs):
                    b = b0 + gi
                    a_ts = []
                    dma_engs = ([nc.sync, nc.scalar, nc.gpsimd]
                                if TUNE.get("multi_eng_dma")
                                else [nc.sync])
                    for s, (a_dt, _, _) in enumerate(streams):
                        a_t = apool.tile([128, NCH * N], _dt(a_dt), tag=f"a{s}")
                        step = NCH * N // a_dma_split
                        for d in range(a_dma_split):
                            eng = dma_engs[b % len(dma_engs)]
                            eng.dma_start(
                                out=a_t[:, d * step:(d + 1) * step],
                                in_=ats[s][b][:, d * step:(d + 1) * step])
                        a_ts.append(a_t)
                    for s, (_, _, y_comps) in enumerate(streams):
                        w = _w(y_comps)
                        for c in range(NCH):
                            nc.tensor.matmul(
                                ps[0:w, gi * N:(gi + 1) * N],
                                y_ts[s][:, (b * NCH + c) * w:
                                        (b * NCH + c + 1) * w],
                                a_ts[s][:, c * N:(c + 1) * N],
                                start=(s == 0 and c == 0),
                                stop=(c == NCH - 1 and (s == 0 or s == last_s)),
                                skip_group_check=True,
                            )
                # add stream 0's 32-row-spaced component bands on VectorE
                # (an op may read PSUM through at most one input, so stage
                # the extra band through SBUF first)
                if n_groups == 1 or TUNE.get("no_combine"):
                    z_tile, z_row = ps, 0
                else:
                    # note: an SBUF x SBUF binary op requires equal base
                    # partitions, and at most one input may be PSUM, so
                    # copy-band-to-SBUF + PSUM-plus-SBUF add is minimal.
                    wN = Gs * N  # ops span only the live batches' columns
                    acc = spool.tile([H, G * N], f32, tag="acc")
                    for k in range(1, n_groups):
                        tmp = spool.tile([H, G * N], f32, tag="tmp")
                        nc.vector.tensor_copy(
                            out=tmp[:, 0:wN],
                            in_=ps[32 * k:32 * k + H, 0:wN])
                        nc.vector.tensor_add(
                            acc[:, 0:wN],
                            ps[0:H, 0:wN] if k == 1 else acc[:, 0:wN],
                            tmp[:, 0:wN])
                    z_tile, z_row = acc, 0
                sc = spool.tile([H, G * N], f32, tag="sc")
                for gi in range(Gs):
                    nc.scalar.activation(
                        sc[:, gi * N:(gi + 1) * N],
                        z_tile[z_row:z_row + H, gi * N:(gi + 1) * N],
                        mybir.ActivationFunctionType.Relu,
                        bias=b1_t[:], scale=1.0,
                        accum_out=out_t[:, b0 + gi:b0 + gi + 1],
                    )

            # the last `tail_singles` batches run as single-batch groups so
            # the post-final-DMA pipeline drain is one short chain, not a
            # full G-batch chain
            tail_singles = TUNE.get("tail_singles", 2)
            tail_start = BPC - tail_singles if G > 1 else BPC

            def emit_all():
                for b0 in range(0, tail_start, G):
                    group_body(b0)
                for b0 in range(tail_start, BPC):
                    group_body(b0, Gs=1)

            if repeat == 1:
                emit_all()
            else:
                # the loop body holds ~256 PE instructions (one IRAM block),
                # so hint the PE's back-edge branch target to avoid an
                # ~3-4us I$-miss stall per iteration in the timing loop
                hints = tuple(getattr(mybir.EngineType, e)
                              for e in TUNE.get("hint", ()))
                with tc.For_i(0, repeat, 1, hint_engines=hints,
                              staggered_reset=TUNE.get("staggered", False)):
                    emit_all()
            nc.sync.dma_start(out=outd[:], in_=out_t[:])

    nc.compile()
    return nc


def _get_nc(strategy=None, repeat=1):
    strategy = strategy or STRATEGY
    key = (strategy, repeat)
    if key not in _BUILT:
        _BUILT[key] = _build(strategy, repeat)
    return _BUILT[key]


def _pack_at(adj):
    """[Bc, N, N] f32 -> A^T packed [Bc, 128, NCH*N] (see _build)."""
    t = adj.reshape(adj.shape[0], N, NCH, 128)  # [b, n, c, p]
    return np.ascontiguousarray(t.transpose(0, 3, 2, 1)).reshape(
        adj.shape[0], 128, NCH * N)


def _pack_y(comps, tight):
    """comps: list of [Bc, N, H] f32 arrays -> [128, Bc*NCH*W]; comp k at
    columns [step*k, step*k+H) of each (b, c) block (step = H if tight,
    else 32 with zero padding)."""
    bc = comps[0].shape[0]
    step = H if tight else 32
    w = step * (len(comps) - 1) + H
    out = np.zeros((128, bc, NCH, w), np.float32)
    for k, y in enumerate(comps):
        # y [b, c, p, j] -> [p, b, c, j]
        out[:, :, :, step * k:step * k + H] = y.reshape(
            bc, NCH, 128, H).transpose(2, 0, 1, 3)
    return out.reshape(128, bc * NCH * w)


def _split(full, dt_name):
    """Return (hi, lo) parts of `full` (f32) for the given storage dtype."""
    np_dt = _np_dt(dt_name)
    hi = full.astype(np_dt)
    lo = (full - hi.astype(np.float32)).astype(np_dt)
    return hi, lo


def _prep_in_maps_f8ms(node_features, adj_matrices, W1, b1):
    import ml_dtypes
    f8 = ml_dtypes.float8_e4m3
    y_full = np.einsum("bnf,fh->bnh", node_features, W1).astype(np.float32)
    colsum = y_full.astype(np.float64).sum(axis=1)  # [B, H]
    b1p_all = (np.asarray(b1, np.float64)[None, :]
               + 0.5 * colsum).astype(np.float32)  # [B, H]
    in_maps = []
    for core in range(N_CORES):
        sl = slice(core * BPC, (core + 1) * BPC)
        d = adj_matrices[sl] - np.float32(0.5)
        at = _pack_at(np.ascontiguousarray(d))  # [BPC, 128, NCH*N]
        if TUNE.get("apack"):
            at8 = np.ascontiguousarray(
                at.astype(ml_dtypes.float8_e4m3).transpose(1, 0, 2)
            ).reshape(128, BPC * NCH * N)
        else:
            at8 = at.reshape(BPC, 128, NCH, N).astype(f8)
        y8 = _pack_y([y_full[sl]], tight=True)  # [128, BPC*NCH*H]
        b1s = np.zeros((32, BPC), np.float32)
        b1s[0:H, :] = b1p_all[sl].T
        in_maps.append({
            "at0": at8,
            "y0": y8.reshape(128, BPC * NCH, H).astype(f8),
            "b1p": b1s,
        })
    return in_maps


def _prep_in_maps(node_features, adj_matrices, W1, b1, strategy):
    if strategy == "f8ms":
        return _prep_in_maps_f8ms(node_features, adj_matrices, W1, b1)
    streams = STRATEGIES[strategy]
    y_full = np.einsum("bnf,fh->bnh", node_features, W1).astype(np.float32)
    b1_col = np.asarray(b1, np.float32).reshape(H, 1)
    in_maps = []
    for core in range(N_CORES):
        sl = slice(core * BPC, (core + 1) * BPC)
        at = _pack_at(np.ascontiguousarray(adj_matrices[sl]))
        y_sh = y_full[sl]
        m = {"b1": b1_col}
        a_parts = {}
        for s, (a_dt, a_part, y_comps) in enumerate(streams):
            if (a_dt, a_part) not in a_parts:
                if a_part == "hi":
                    a_parts[(a_dt, "hi")] = at.astype(_np_dt(a_dt))
                else:
                    hi = at.astype(_np_dt(streams[0][0]))
                    a_parts[(a_dt, "lo")] = (
                        at - hi.astype(np.float32)).astype(_np_dt(a_dt))
            m[f"at{s}"] = a_parts[(a_dt, a_part)]
            comps = []
            for (y_dt, y_part) in y_comps:
                hi, lo = _split(y_sh, y_dt)
                comps.append((hi if y_part == "hi" else lo).astype(np.float32))
            m[f"y{s}"] = _pack_y(comps, TUNE.get("tight_y", True)).astype(
                _np_dt(y_comps[0][0]))
        in_maps.append(m)
    return in_maps


def _finish(results, W2, b2):
    # results[c]["out"]: [H, BPC]; colsum[b, j] = sum_n relu(Z + b1)[n, j]
    cols = np.stack([r["out"] for r in results])  # [8, H, BPC]
    colsum = cols.transpose(0, 2, 1).reshape(B, H).astype(np.float32)
    out = colsum @ np.asarray(W2, np.float32) + N * np.asarray(b2, np.float32)
    return out.astype(np.float32)


def _finish_f8ms(results, W2, b2):
    colsum = np.empty((B, H), np.float32)
    for core, r in enumerate(results):
        o = r["out"].reshape(32, BPC, 6)  # [32, b, stat]
        if TUNE.get("act_accum"):
            colsum[core * BPC:(core + 1) * BPC] = o[0:H, :, 0].T
        else:
            # bn_stats: sum = 256 * (mean(even) + mean(odd))
            colsum[core * BPC:(core + 1) * BPC] = (
                256.0 * (o[0:H, :, 1] + o[0:H, :, 4]).T)
    out = colsum @ np.asarray(W2, np.float32) + N * np.asarray(b2, np.float32)
    return out.astype(np.float32)


def kernel(node_features, adj_matrices, W1, b1, W2, b2):
    node_features = np.asarray(node_features, np.float32)
    adj_matrices = np.asarray(adj_matrices, np.float32)
    nc = _get_nc()
    in_maps = _prep_in_maps(node_features, adj_matrices, W1, b1, STRATEGY)
    res = run_bass_kernel_spmd(nc, in_maps, core_ids=list(range(N_CORES)))
    if STRATEGY == "f8ms":
        return _finish_f8ms(res.results, W2, b2)
    return _finish(res.results, W2, b2)

